# revision 40
# baseline (speedup 1.0000x reference)
# Trainium2 Bass kernel for nn_Attention3 (unnormalized linear attention).
#
# Math: e_i = x @ W_i.T + b_i (i=1,2,3);  out = sigmoid((e1 @ e2.T @ e3) @ WO.T + bO)
# Since there is no softmax, (e1 @ e2.T) @ e3 == e1 @ (e2.T @ e3) where
# KV = e2.T @ e3 is only [64, 64].
#
# Sharding: the flattened [B*S, 512] = [16384, 512] rows are split into 8
# chunks of 2048 rows (cores 0-3 <- batch 0, cores 4-7 <- batch 1).  Every
# core streams its WHOLE batch to build the full KV^T = e3.T @ e2 locally --
# redundant compute, but fully deterministic: no collectives or cross-core
# synchronization (measured ncfw AllGather latency on this setup is 25-100us
# with heavy per-core skew, far worse than the extra DMA).  Each core's OWN
# 2048 rows are ordered first in its input so e1 and the output stage run on
# chunks 0-3 with uniform (SPMD) code.
#
# Engine split per chunk (v2): PE does e23T (4xN=512) + 4 transposes + 4 KV
# accumulations; DVE does the e23 bias-add+cast (PSUM f32 -> SBUF f16,
# ~0.75us); Scalar does the post-transpose PSUM f16 -> SBUF copy (~0.72us).
# All three sit just under the PE cadence (~1.25us/chunk) so the chunk
# pipeline is PE/DMA-co-limited instead of DVE-chained.
#
# Phase C (v2): z = e1 @ M matmuls write f16 PSUM (two [128,512] z-tiles per
# 2KB bank); the sigmoid runs on Scalar DIRECTLY from PSUM in 4-tile batches
# (no DVE casts -- ACTIVATE rate is dtype-independent 1x, so pre-casting
# buys nothing and the old DVE CAST chain at 681ns/tile was the tail
# bottleneck).  The PE stays busy through the A->C transition so the HAM
# clock gate keeps the 2.4 GHz clock (baseline ran the whole tail at 1.2).
#
# Precision: x is cast to float16 on the HOST, halving HBM read traffic.
# All matmuls run f16 operands / fp32 PSUM (z-tiles drain to f16 PSUM: the
# pre-sigmoid values |z|<~60 round at ~5e-4 rel).  Output written f16.
#
# Layouts: x arrives host-transposed and pre-tiled as [128, chunk, 2048] f16
# so every DMA reads contiguous 4 KiB per partition; the output is written
# back partition-major ([128, rows*4] f16) and un-permuted on the host.
# All weights/biases arrive packed in one [128, 1922] f16 blob used as
# views; the first 896 cols (W1/W23/identity) ride the fast sync HWDGE ring
# so they land before the first e23T matmul, the rest (WO/bO) ride gpsimd.

import numpy as np

import concourse.bass as bass
import concourse.mybir as mybir
import concourse.tile as tile
from concourse import bacc
from concourse.bass_utils import run_bass_kernel_spmd
from concourse.vector_clock import ScopedClock


class _FastExitTileContext(tile.TileContext):
    """TileContext whose exit skips the end-of-run semaphore range clear.

    The stock ``_drain_and_barrier`` emits a DMA-queue reset plus an
    EVENT_SEMAPHORE_RANGE_CLEAR over every semaphore the kernel used
    (~250 here).  The HW applies those writes serially at ~28ns each,
    which puts ~7us of pure bookkeeping at the very end of the measured
    execution window.  The kernel PREAMBLE (Bacc.reset) already clears the
    whole bass-managed semaphore range at the start of every execution, so
    the end-of-run clear is redundant for back-to-back executions of this
    NEFF.  We keep the global-clock drain (all compute/DMA settled before
    the final NOTIFY) and the engine barrier."""

    def _drain_and_barrier(self, tick_clock, wait_clock):
        # The stock exit also attaches a wait for the GLOBAL vector clock
        # (every sem's final value) to this drain; generate_event_semaphores
        # then splits that into ~100 serial EVENT_SEMAPHORE instructions
        # (~6us).  Every producer in this kernel is transitively ordered
        # before the final output DMAs via data deps, and each engine's
        # in-order stream ends with its own work, so drain+barrier alone
        # settles everything the final NOTIFY must cover.
        self.nc.sync.drain()
        popped = self.nc._tile_sem_poison_stack.pop()
        assert popped is self._sem_poison

BATCH = 2
SEQ = 8192
DIN = 512
DE = 64
N_CORES = 8
ROWS = (BATCH * SEQ) // N_CORES  # 2048 output rows per core
OWN0 = 2  # stream position of the first own chunk (see _front_half)

# const blob layout (free-dim offsets, f16, [128, NB])
_OFF_W1T = 0          # [128, 4, 64]   w1t rearranged (kt p) d -> p kt d
_OFF_W23T = 256       # [128, 4, 128]  w23t rearranged
_OFF_IDENT = 768      # [128, 128]     identity
_OFF_B23 = 896        # [128, 1]       b2|b3 (per-partition)
_OFF_WOT = 897        # [64, 512]      WO.T (rows 0..63)
_OFF_B1 = 1409        # [64, 1]        b1 (rows 0..63)
_OFF_BO = 1410        # [1, 512]       bO (row 0)
_NB = 1922

TRACE = False
TRACE_KWARGS = {}
LAST_RESULT = None

_NC_CACHE = {}


def build_nc(rows=ROWS, n_cores=N_CORES):
    f32 = mybir.dt.float32
    f16 = mybir.dt.float16

    group = n_cores // 2  # cores per batch
    assert rows % 512 == 0
    own_chunks = rows // 512
    n_chunks = own_chunks * group  # whole batch streamed per core

    nc = bacc.Bacc(
        None,
        target_bir_lowering=False,
        debug=False,
        num_devices=n_cores,
    )

    xt = nc.dram_tensor("xt", [128, n_chunks * 2048], f16, kind="ExternalInput")
    wconst = nc.dram_tensor("wconst", [128, _NB], f16, kind="ExternalInput")
    # b2|b3 (col 0, 128 rows) and b1 (col 1, rows 0..63) — DVE tensor_scalar
    # requires a float32 scalar operand, so these ride outside the f16 blob.
    bias32 = nc.dram_tensor("bias32", [128, 2], f32, kind="ExternalInput")
    out = nc.dram_tensor("out", [128, rows * 4], f16, kind="ExternalOutput")

    xt_t = xt.ap().rearrange("p (j f) -> p j f", f=2048)  # [128, n_chunks, 2048]

    with _FastExitTileContext(nc) as tc:
        with (
            tc.tile_pool(name="consts", bufs=1) as consts,
            tc.tile_pool(name="persist", bufs=1) as persist,
            tc.tile_pool(name="small", bufs=1) as small,
        ):
            blob = consts.tile([128, _NB], f16)
            # W23T gates the very first e23T matmul: it goes out FIRST on the
            # sync ring, ahead of the x quarters.  W1T+identity ride the
            # scalar ring in parallel.  The tail-only WO.T/bO half is held
            # back (dep added below) so it does not steal early bandwidth.
            nc.sync.dma_start(
                out=blob[:, _OFF_W23T:_OFF_B23],
                in_=wconst.ap()[:, _OFF_W23T:_OFF_B23],
            )
            nc.scalar.dma_start(
                out=blob[:, :_OFF_W23T], in_=wconst.ap()[:, :_OFF_W23T]
            )
            biases = consts.tile([128, 2], f32)
            nc.gpsimd.dma_start(out=biases, in_=bias32.ap())
            blob_rest = nc.gpsimd.dma_start(
                out=blob[:, _OFF_B23:], in_=wconst.ap()[:, _OFF_B23:]
            )

            sb_w1t = blob[:, _OFF_W1T : _OFF_W1T + 256].rearrange(
                "p (kt d) -> p kt d", kt=4
            )
            sb_w23t = blob[:, _OFF_W23T : _OFF_W23T + 512].rearrange(
                "p (kt d) -> p kt d", kt=4
            )
            sb_wot = blob[:DE, _OFF_WOT : _OFF_WOT + DIN]
            identity = blob[:, _OFF_IDENT : _OFF_IDENT + 128]
            sb_b23 = biases[:, 0:1]
            sb_b1 = biases[:DE, 1:2]
            sb_bo = blob[:1, _OFF_BO : _OFF_BO + DIN]

            # HAM warm-up operand: zeroed early by the (preamble-fast) DVE so
            # keep-alive matmuls can start ~6.5us, bridging to the first x
            # quarter-chunk (~7.8us) and warming the PE clock by ~10us.
            warm_sb = consts.tile([128, 512], f16)
            nc.vector.memset(warm_sb, 0.0)

            # e1^T for the own rows, with a row of ones at partition DE so the
            # final matmul folds in the output bias (lhsT K = DE+1).
            e1t = persist.tile([128, rows], f16)
            nc.vector.memset(e1t[DE : DE + 1, :], 1.0)
            # M = KV @ WO.T in rows 0..63, bO in row DE (bO row copied at the
            # phase A tail -- it depends on the delayed blob half, and an
            # early copy would head-of-line-block the DVE queue on it).
            mmat = persist.tile([128, DIN], f16)

            # ---- Phase A: stream the whole batch, e2|e3 -> KV^T; e1 for
            # the own chunks (j < own_chunks) ----
            # Software-pipelined by one chunk: per iteration PE runs
            # [transpose(j-1), e23T(j), KV(j-1), e1(j)] -- each op's producer
            # on another engine finished an iteration ago.
            # ALL PSUM pools (incl. the KV accumulators) live inside this
            # block so phase C gets the full 8 banks for sigmoid batching.
            with (
                tc.tile_pool(name="xf", bufs=6) as xfp,
                tc.tile_pool(name="e23tps", bufs=3, space="PSUM") as e23tpsp,
                tc.tile_pool(name="e23tsb", bufs=4) as e23tsbp,
                tc.tile_pool(name="trps", bufs=2, space="PSUM") as trpsp,
                tc.tile_pool(name="e23n", bufs=3) as e23np,
                tc.tile_pool(name="e1ps", bufs=1, space="PSUM") as e1psp,
                tc.tile_pool(name="kvps", bufs=1, space="PSUM") as kvps,
                tc.tile_pool(name="kvbps", bufs=1, space="PSUM") as kvbps,
            ):
                # Two KV accumulators in separate PSUM banks: kvt_ps (chunks
                # 0..n-3) and kvb_ps (last two chunks).  Splitting KV lets
                # M = KV @ WO.T start before the final chunk arrives.
                kvt_ps = kvps.tile([DE, DE], f32)
                kvb_ps = kvbps.tile([DE, DE], f32)
                # One bank, two partition-disjoint regions: e1 accumulates on
                # partitions 0..63, keep-alive dummies write partitions
                # 64..127.
                e1kv = e1psp.tile([128, 512], f32)

                # The Tile scheduler reorders same-engine work by its own
                # cost model, which bunches the e23T groups and serializes
                # the TR->copy->KV chain.  Pin every engine's stream to
                # program order with order-only (sync=False) deps.
                _prev = {}

                def _pin(key, bi):
                    p = _prev.get(key)
                    if p is not None:
                        tile.add_dep_helper(
                            bi.ins, p.ins, sync=False, reason="pin order"
                        )
                    _prev[key] = bi
                    return bi

                def _mm(*args, **kwargs):
                    return _pin("pe", nc.tensor.matmul(*args, **kwargs))

                def _tr(*args, **kwargs):
                    return _pin("pe", nc.tensor.transpose(*args, **kwargs))

                def _dve(bi):
                    return _pin("dve", bi)

                def _act(bi):
                    return _pin("act", bi)

                def _dummy_mms(k):
                    for _ in range(k):
                        _mm(e1kv[DE:, :], lhsT=warm_sb[:, :DE], rhs=warm_sb)

                # Load the sigmoid activation table up front (the junk
                # sigmoid also primes the Scalar pipeline).
                warm_o = small.tile([1, 4], f16)
                _act(nc.scalar.activation(
                    warm_o, warm_sb[:1, :4], mybir.ActivationFunctionType.Sigmoid
                ))
                # One CONTIGUOUS keep-alive burst (~4.3us at the cold clock).
                # The HAM clock gate only un-throttles after a full
                # 4096-cycle window of sustained PE activity; fragmented
                # fills (interleaved with data waits) leave the PE at
                # 1.2 GHz for 15-22us.  The burst also bridges engine-ready
                # (~7.4us) to weights+first-data (~8.7us).
                _dummy_mms(14)
                state = {}  # j -> (e23t_sb | e23n) between pipeline stages

                def _transpose_part(j):
                    # transpose e23T back to natural layout (batched into one
                    # PSUM bank); the drain copy runs on the Scalar engine --
                    # DVE keeps only the bias-add so neither engine exceeds
                    # the ~1.25us chunk cadence.
                    e23t_sb = state.pop(j)
                    tr_ps = trpsp.tile([128, 512], f16)
                    for t in range(4):
                        _tr(
                            tr_ps[:, t * 128 : (t + 1) * 128],
                            e23t_sb[:, t * 128 : (t + 1) * 128],
                            identity[:, :],
                        )
                    e23n = e23np.tile([128, 512], f16)
                    _act(nc.scalar.copy(e23n, tr_ps))
                    state[j] = e23n

                def _kv_part(j):
                    # accumulate KV^T = e3^T @ e2.  The last two chunks go to
                    # the second accumulator so the first (dominant) KV part
                    # can head into M early.
                    e23n = state.pop(j)
                    last2 = j >= n_chunks - 2
                    dst = kvb_ps if last2 else kvt_ps
                    lo = (n_chunks - 2) * 4 if last2 else 0
                    hi = 4 * n_chunks - 1 if last2 else (n_chunks - 2) * 4 - 1
                    for t in range(4):
                        tt = j * 4 + t
                        _mm(
                            dst,
                            lhsT=e23n[:, t * 128 + DE : (t + 1) * 128],
                            rhs=e23n[:, t * 128 : t * 128 + DE],
                            start=(tt == lo),
                            stop=(tt == hi),
                        )

                def _front_half(j, xr):
                    # e23T = [W2;W3] @ x^T  -> [128, 512] (d on partitions)
                    e23t_ps = e23tpsp.tile([128, 512], f32)
                    for kt in range(4):
                        _mm(
                            e23t_ps,
                            lhsT=sb_w23t[:, kt, :],
                            rhs=xr[:, kt, :],
                            start=(kt == 0),
                            stop=(kt == 3),
                        )
                    # bias add + f16 cast on DVE
                    e23t_sb = e23tsbp.tile([128, 512], f16)
                    _dve(nc.vector.tensor_scalar_add(e23t_sb, e23t_ps, sb_b23))
                    state[j] = e23t_sb

                    # e1T = W1 @ x^T (+b1) for the rows this core outputs.
                    # Own chunks ride at positions 2..5: early enough that e1
                    # never lands on the post-stream tail, late enough that
                    # the cold-clock window (first ~2 chunks) only covers
                    # single-size chunks.
                    if OWN0 <= j < OWN0 + own_chunks:
                        jo = j - OWN0
                        e1_ps = e1kv[:DE, :]
                        for kt in range(4):
                            _mm(
                                e1_ps,
                                lhsT=sb_w1t[:, kt, :],
                                rhs=xr[:, kt, :],
                                start=(kt == 0),
                                stop=(kt == 3),
                            )
                        _dve(nc.vector.tensor_scalar_add(
                            e1t[:DE, jo * 512 : (jo + 1) * 512], e1_ps, sb_b1
                        ))

                # Single-chunk DMAs throughout: DMA completion is
                # all-or-nothing per instruction, so a 4-chunk (2MB) group
                # would stall the consumer of its FIRST chunk for an extra
                # ~4.4us.  512KB transfers still run at ~350 GB/s.
                # (Splitting the stream across the sync+scalar HWDGE rings
                # was tried and measured ~5us WORSE: the scalar-ring issues
                # interleave with the e23n copy chain.)
                schedule = [1, 1] + [2] * ((n_chunks - 2) // 2)
                assert sum(schedule) == n_chunks
                # M = KV @ WO.T accumulates in the e1 region of the shared
                # bank -- e1 is long finished (own chunks come first).
                mm_ps = e1kv[:DE, :]
                j = 0
                for g in schedule:
                    xf = xfp.tile([128, g, 2048], f16, tag=f"xf{g}")
                    if j == 0:
                        # quarter-granularity leading DMAs: the first e23T
                        # matmul only needs the first 512 columns
                        for kt in range(4):
                            nc.sync.dma_start(
                                out=xf[:, 0, kt * 512 : (kt + 1) * 512],
                                in_=xt_t[:, 0, kt * 512 : (kt + 1) * 512],
                            )
                    else:
                        nc.sync.dma_start(
                            out=xf[:, :g, :], in_=xt_t[:, j : j + g, :]
                        )
                    for j2 in range(g):
                        xr = xf[:, j2, :].rearrange("p (kt s) -> p kt s", kt=4)
                        # matmuls first, then transpose(j-1), then KV(j-2):
                        # the PE is strictly in-order, so the KV stage needs
                        # TWO chunks of lag for the transpose -> Scalar-copy
                        # round trip to finish off the critical path.
                        _front_half(j, xr)
                        if j >= 1:
                            _transpose_part(j - 1)
                        if j >= 2:
                            _kv_part(j - 2)
                        if j == 3:
                            # release the tail-only blob half only after the
                            # early ramp: it must not compete with W23T / the
                            # first x chunks for DMA bandwidth
                            tile.add_dep_helper(
                                blob_rest.ins, _prev["pe"].ins,
                                sync=True, reason="delay tail blob",
                            )
                        j += 1
                # Tail peel.  KV runs at lag 2, so kvt (chunks 0..n-3)
                # completed inside the loop; its SBUF copy overlaps the
                # remaining PE work so M part 1 is ready just in time.
                kvt_r = small.tile([DE, DE], f16)
                _dve(nc.vector.tensor_copy(kvt_r, kvt_ps))
                _transpose_part(n_chunks - 1)
                _kv_part(n_chunks - 2)
                _mm(mm_ps, lhsT=kvt_r, rhs=sb_wot, start=True, stop=False)
                _kv_part(n_chunks - 1)
                kvb_r = small.tile([DE, DE], f16)
                _dve(nc.vector.tensor_copy(kvb_r, kvb_ps))
                _dve(nc.vector.tensor_copy(mmat[DE : DE + 1, :], sb_bo))
                _dummy_mms(2)
                _mm(mm_ps, lhsT=kvb_r, rhs=sb_wot, start=False, stop=True)
                _dummy_mms(2)
                _dve(nc.vector.tensor_copy(mmat[:DE, :], mm_ps))

            # ---- Phase C: out = sigmoid(e1 @ M + bO) ----
            # z-tiles land as f32 in PSUM (one bank each, 4 banks per pool
            # tile); the sigmoid runs on Scalar DIRECTLY from PSUM over
            # 4-tile batches -- no DVE casts (ACTIVATE is 1x rate regardless
            # of dtype, so the old CAST+ACT chain only added DVE
            # serialization).  The PE issues all 16 z matmuls back-to-back
            # at the warm clock.
            with (
                tc.tile_pool(name="zps", bufs=2, space="PSUM") as zpsp,
                tc.tile_pool(name="osb", bufs=3) as osbp,
            ):
                out_flat = out.ap()
                nbat = rows // 128 // 4  # 4 z-tiles (= 4 PSUM banks) per batch
                for p in range(nbat):
                    z_ps = zpsp.tile([128, 4, DIN], f32, tag="zps")
                    for i in range(4):
                        t = 4 * p + i
                        nc.tensor.matmul(
                            z_ps[:, i, :],
                            lhsT=e1t[: DE + 1, t * 128 : (t + 1) * 128],
                            rhs=mmat[: DE + 1, :],
                        )
                    osb = osbp.tile([128, 4, DIN], f16, tag="osb")
                    # all out-DMAs ride the (idle) sync ring -- issuing from
                    # the scalar ring put a ~650ns DMA-issue in the middle of
                    # the sigmoid chain, which is the tail critical path.
                    # The last batch's sigmoid is split 3+1 so the final
                    # flush after the very last sigmoid is only ~128KB.
                    if p < nbat - 1:
                        nc.scalar.activation(
                            osb, z_ps, mybir.ActivationFunctionType.Sigmoid
                        )
                        nc.sync.dma_start(
                            out=out_flat[:, p * 2048 : (p + 1) * 2048],
                            in_=osb,
                        )
                    else:
                        nc.scalar.activation(
                            osb[:, :3, :], z_ps[:, :3, :],
                            mybir.ActivationFunctionType.Sigmoid,
                        )
                        nc.sync.dma_start(
                            out=out_flat[:, p * 2048 : p * 2048 + 1536],
                            in_=osb[:, 0:3, :],
                        )
                        nc.scalar.activation(
                            osb[:, 3:, :], z_ps[:, 3:, :],
                            mybir.ActivationFunctionType.Sigmoid,
                        )
                        nc.sync.dma_start(
                            out=out_flat[:, p * 2048 + 1536 : (p + 1) * 2048],
                            in_=osb[:, 3:4, :],
                        )
    nc.compile()
    return nc


def make_wconst(W1, b1, W2, b2, W3, b3, WO, bO):
    blob = np.zeros((128, _NB), np.float16)
    w1t = np.asarray(W1, np.float16).T.reshape(4, 128, DE)  # (kt, p, d)
    blob[:, _OFF_W1T : _OFF_W1T + 256] = (
        w1t.transpose(1, 0, 2).reshape(128, 4 * DE)
    )
    w23t = np.concatenate(
        [np.asarray(W2, np.float16).T, np.asarray(W3, np.float16).T], axis=1
    ).reshape(4, 128, 2 * DE)
    blob[:, _OFF_W23T : _OFF_W23T + 512] = (
        w23t.transpose(1, 0, 2).reshape(128, 8 * DE)
    )
    blob[:, _OFF_IDENT : _OFF_IDENT + 128] = np.eye(128, dtype=np.float16)
    blob[:, _OFF_B23] = np.concatenate(
        [np.asarray(b2, np.float16), np.asarray(b3, np.float16)]
    )
    blob[:DE, _OFF_WOT : _OFF_WOT + DIN] = np.asarray(WO, np.float16).T
    blob[:DE, _OFF_B1] = np.asarray(b1, np.float16)
    blob[0, _OFF_BO : _OFF_BO + DIN] = np.asarray(bO, np.float16)
    return blob


def _tile_rows(xc):
    """[rows, 512] f16 -> [128, (rows/512)*2048] in (p, chunk, kt, s) order."""
    n = xc.shape[0] // 512
    return np.ascontiguousarray(
        xc.reshape(n, 512, 4, 128).transpose(3, 0, 2, 1)
    ).reshape(128, n * 2048)


def make_in_maps(x, W1, b1, W2, b2, W3, b3, WO, bO, rows=ROWS, n_cores=N_CORES):
    x = np.asarray(x, dtype=np.float32).astype(np.float16)
    total = x.shape[0] * x.shape[1]
    xf = x.reshape(total, DIN)
    blob = make_wconst(W1, b1, W2, b2, W3, b3, WO, bO)
    bvec = np.zeros((128, 2), np.float32)
    bvec[:, 0] = np.concatenate([np.asarray(b2, np.float32), np.asarray(b3, np.float32)])
    bvec[:DE, 1] = np.asarray(b1, np.float32)
    group = n_cores // 2
    batch_rows = rows * group
    in_maps = []
    for c in range(n_cores):
        b, q = divmod(c, group)
        xb = xf[b * batch_rows : (b + 1) * batch_rows]  # full batch of this core
        own = xb[q * rows : (q + 1) * rows]
        rest = np.concatenate([xb[: q * rows], xb[(q + 1) * rows :]], axis=0)
        cut = OWN0 * 512  # own chunks sit at stream positions OWN0..OWN0+3
        m = {
            "wconst": blob,
            "bias32": bvec,
            "xt": np.concatenate(
                [
                    _tile_rows(rest[:cut]),
                    _tile_rows(own),
                    _tile_rows(rest[cut:]),
                ],
                axis=1,
            ),
        }
        in_maps.append(m)
    return in_maps


def unshard_out(o, rows=ROWS):
    # o: [128, rows*4] f16 laid out (p, j, t, o) -> rows j*512 + t*128 + p
    n_chunks = rows // 512
    return (
        o.astype(np.float32)
        .reshape(128, n_chunks, 4, DIN)
        .transpose(1, 2, 0, 3)
        .reshape(rows, DIN)
    )


def kernel(x, W1, b1, W2, b2, W3, b3, WO, bO):
    global LAST_RESULT
    if "nc" not in _NC_CACHE:
        _NC_CACHE["nc"] = build_nc()
    nc = _NC_CACHE["nc"]
    in_maps = make_in_maps(x, W1, b1, W2, b2, W3, b3, WO, bO)
    res = run_bass_kernel_spmd(
        nc,
        in_maps,
        core_ids=list(range(N_CORES)),
        trace=TRACE,
        **TRACE_KWARGS,
    )
    LAST_RESULT = res
    full = np.concatenate(
        [unshard_out(res.results[c]["out"]) for c in range(N_CORES)], axis=0
    )  # [16384, 512] f32
    return full.reshape(BATCH, SEQ, DIN)


# revision 41
# speedup vs baseline: 1.1093x; 1.1093x over previous
# Trainium2 Bass kernel for nn_Attention3 (unnormalized linear attention).
#
# Math: e_i = x @ W_i.T + b_i (i=1,2,3);  out = sigmoid((e1 @ e2.T @ e3) @ WO.T + bO)
# Since there is no softmax, (e1 @ e2.T) @ e3 == e1 @ (e2.T @ e3) where
# KV = e2.T @ e3 is only [64, 64].
#
# Sharding: the flattened [B*S, 512] = [16384, 512] rows are split into 8
# chunks of 2048 rows (cores 0-3 <- batch 0, cores 4-7 <- batch 1).  Every
# core streams its WHOLE batch to build the full KV^T = e3.T @ e2 locally --
# redundant compute, but fully deterministic: no collectives or cross-core
# synchronization (measured ncfw AllGather latency on this setup is 25-100us
# with heavy per-core skew, far worse than the extra DMA).  Each core's OWN
# 2048 rows are ordered first in its input so e1 and the output stage run on
# chunks 0-3 with uniform (SPMD) code.
#
# Engine split per chunk (v2): PE does e23T (4xN=512) + 4 transposes + 4 KV
# accumulations; DVE does the e23 bias-add+cast (PSUM f32 -> SBUF f16,
# ~0.75us); Scalar does the post-transpose PSUM f16 -> SBUF copy (~0.72us).
# All three sit just under the PE cadence (~1.25us/chunk) so the chunk
# pipeline is PE/DMA-co-limited instead of DVE-chained.
#
# Phase C (v2): z = e1 @ M matmuls write f16 PSUM (two [128,512] z-tiles per
# 2KB bank); the sigmoid runs on Scalar DIRECTLY from PSUM in 4-tile batches
# (no DVE casts -- ACTIVATE rate is dtype-independent 1x, so pre-casting
# buys nothing and the old DVE CAST chain at 681ns/tile was the tail
# bottleneck).  The PE stays busy through the A->C transition so the HAM
# clock gate keeps the 2.4 GHz clock (baseline ran the whole tail at 1.2).
#
# Precision: x is cast to float16 on the HOST, halving HBM read traffic.
# All matmuls run f16 operands / fp32 PSUM (z-tiles drain to f16 PSUM: the
# pre-sigmoid values |z|<~60 round at ~5e-4 rel).  Output written f16.
#
# Layouts: x arrives host-transposed and pre-tiled as [128, chunk, 2048] f16
# so every DMA reads contiguous 4 KiB per partition; the output is written
# back partition-major ([128, rows*4] f16) and un-permuted on the host.
# All weights/biases arrive packed in one [128, 1922] f16 blob used as
# views; the first 896 cols (W1/W23/identity) ride the fast sync HWDGE ring
# so they land before the first e23T matmul, the rest (WO/bO) ride gpsimd.

import numpy as np

import concourse.bass as bass
import concourse.mybir as mybir
import concourse.tile as tile
from concourse import bacc
from concourse.bass_utils import run_bass_kernel_spmd
from concourse.vector_clock import ScopedClock


class _FastExitTileContext(tile.TileContext):
    """TileContext whose exit skips the end-of-run semaphore range clear.

    The stock ``_drain_and_barrier`` emits a DMA-queue reset plus an
    EVENT_SEMAPHORE_RANGE_CLEAR over every semaphore the kernel used
    (~250 here).  The HW applies those writes serially at ~28ns each,
    which puts ~7us of pure bookkeeping at the very end of the measured
    execution window.  The kernel PREAMBLE (Bacc.reset) already clears the
    whole bass-managed semaphore range at the start of every execution, so
    the end-of-run clear is redundant for back-to-back executions of this
    NEFF.  We keep the global-clock drain (all compute/DMA settled before
    the final NOTIFY) and the engine barrier."""

    def _drain_and_barrier(self, tick_clock, wait_clock):
        # The stock exit also attaches a wait for the GLOBAL vector clock
        # (every sem's final value) to this drain; generate_event_semaphores
        # then splits that into ~100 serial EVENT_SEMAPHORE instructions
        # (~6us).  Every producer in this kernel is transitively ordered
        # before the final output DMAs via data deps, and each engine's
        # in-order stream ends with its own work, so drain+barrier alone
        # settles everything the final NOTIFY must cover.
        self.nc.sync.drain()
        popped = self.nc._tile_sem_poison_stack.pop()
        assert popped is self._sem_poison

BATCH = 2
SEQ = 8192
DIN = 512
DE = 64
N_CORES = 8
ROWS = (BATCH * SEQ) // N_CORES  # 2048 output rows per core
OWN0 = 2  # stream position of the first own chunk (see _front_half)

# const blob layout (free-dim offsets, f16, [128, NB])
_OFF_W1T = 0          # [128, 4, 64]   w1t rearranged (kt p) d -> p kt d
_OFF_W23T = 256       # [128, 4, 128]  w23t rearranged
_OFF_IDENT = 768      # [128, 128]     identity
_OFF_B23 = 896        # [128, 1]       b2|b3 (per-partition)
_OFF_WOT = 897        # [64, 512]      WO.T (rows 0..63)
_OFF_B1 = 1409        # [64, 1]        b1 (rows 0..63)
_OFF_BO = 1410        # [1, 512]       bO (row 0)
_NB = 1922

TRACE = False
TRACE_KWARGS = {}
LAST_RESULT = None

_NC_CACHE = {}


def build_nc(rows=ROWS, n_cores=N_CORES):
    f32 = mybir.dt.float32
    f16 = mybir.dt.float16

    group = n_cores // 2  # cores per batch
    assert rows % 512 == 0
    own_chunks = rows // 512
    n_chunks = own_chunks * group  # whole batch streamed per core

    nc = bacc.Bacc(
        None,
        target_bir_lowering=False,
        debug=False,
        num_devices=n_cores,
    )

    xt = nc.dram_tensor("xt", [128, n_chunks * 2048], f16, kind="ExternalInput")
    wconst = nc.dram_tensor("wconst", [128, _NB], f16, kind="ExternalInput")
    # b2|b3 (col 0, 128 rows) and b1 (col 1, rows 0..63) — DVE tensor_scalar
    # requires a float32 scalar operand, so these ride outside the f16 blob.
    bias32 = nc.dram_tensor("bias32", [128, 2], f32, kind="ExternalInput")
    out = nc.dram_tensor("out", [128, rows * 4], f16, kind="ExternalOutput")

    xt_t = xt.ap().rearrange("p (j f) -> p j f", f=2048)  # [128, n_chunks, 2048]

    with _FastExitTileContext(nc) as tc:
        with (
            tc.tile_pool(name="consts", bufs=1) as consts,
            tc.tile_pool(name="persist", bufs=1) as persist,
            tc.tile_pool(name="small", bufs=1) as small,
        ):
            blob = consts.tile([128, _NB], f16)
            # W23T gates the very first e23T matmul: it goes out FIRST on the
            # sync ring, ahead of the x quarters.  W1T+identity ride the
            # scalar ring in parallel.  The tail-only WO.T/bO half is held
            # back (dep added below) so it does not steal early bandwidth.
            nc.sync.dma_start(
                out=blob[:, _OFF_W23T:_OFF_B23],
                in_=wconst.ap()[:, _OFF_W23T:_OFF_B23],
            )
            nc.scalar.dma_start(
                out=blob[:, :_OFF_W23T], in_=wconst.ap()[:, :_OFF_W23T]
            )
            biases = consts.tile([128, 2], f32)
            nc.gpsimd.dma_start(out=biases, in_=bias32.ap())
            blob_rest = nc.gpsimd.dma_start(
                out=blob[:, _OFF_B23:], in_=wconst.ap()[:, _OFF_B23:]
            )

            sb_w1t = blob[:, _OFF_W1T : _OFF_W1T + 256].rearrange(
                "p (kt d) -> p kt d", kt=4
            )
            sb_w23t = blob[:, _OFF_W23T : _OFF_W23T + 512].rearrange(
                "p (kt d) -> p kt d", kt=4
            )
            sb_wot = blob[:DE, _OFF_WOT : _OFF_WOT + DIN]
            identity = blob[:, _OFF_IDENT : _OFF_IDENT + 128]
            sb_b23 = biases[:, 0:1]
            sb_b1 = biases[:DE, 1:2]
            sb_bo = blob[:1, _OFF_BO : _OFF_BO + DIN]

            # HAM warm-up operand: zeroed early by the (preamble-fast) DVE so
            # keep-alive matmuls can start ~6.5us, bridging to the first x
            # quarter-chunk (~7.8us) and warming the PE clock by ~10us.
            warm_sb = consts.tile([128, 512], f16)
            nc.vector.memset(warm_sb, 0.0)

            # e1^T for the own rows, with a row of ones at partition DE so the
            # final matmul folds in the output bias (lhsT K = DE+1).
            e1t = persist.tile([128, rows], f16)
            nc.vector.memset(e1t[DE : DE + 1, :], 1.0)
            # M = KV @ WO.T in rows 0..63, bO in row DE (bO row copied at the
            # phase A tail -- it depends on the delayed blob half, and an
            # early copy would head-of-line-block the DVE queue on it).
            mmat = persist.tile([128, DIN], f16)

            # ---- Phase A: stream the whole batch, e2|e3 -> KV^T; e1 for
            # the own chunks (j < own_chunks) ----
            # Software-pipelined by one chunk: per iteration PE runs
            # [transpose(j-1), e23T(j), KV(j-1), e1(j)] -- each op's producer
            # on another engine finished an iteration ago.
            # ALL PSUM pools (incl. the KV accumulators) live inside this
            # block so phase C gets the full 8 banks for sigmoid batching.
            with (
                tc.tile_pool(name="xf", bufs=6) as xfp,
                tc.tile_pool(name="e23tps", bufs=3, space="PSUM") as e23tpsp,
                tc.tile_pool(name="e23tsb", bufs=4) as e23tsbp,
                tc.tile_pool(name="trps", bufs=2, space="PSUM") as trpsp,
                tc.tile_pool(name="e23n", bufs=3) as e23np,
                tc.tile_pool(name="e1ps", bufs=1, space="PSUM") as e1psp,
                tc.tile_pool(name="kvps", bufs=1, space="PSUM") as kvps,
                tc.tile_pool(name="kvbps", bufs=1, space="PSUM") as kvbps,
            ):
                # Two KV accumulators in separate PSUM banks: kvt_ps (chunks
                # 0..n-3) and kvb_ps (last two chunks).  Splitting KV lets
                # M = KV @ WO.T start before the final chunk arrives.
                kvt_ps = kvps.tile([DE, DE], f32)
                kvb_ps = kvbps.tile([DE, DE], f32)
                # One bank, two partition-disjoint regions: e1 accumulates on
                # partitions 0..63, keep-alive dummies write partitions
                # 64..127.
                e1kv = e1psp.tile([128, 512], f32)

                # The Tile scheduler reorders same-engine work by its own
                # cost model, which bunches the e23T groups and serializes
                # the TR->copy->KV chain.  Pin every engine's stream to
                # program order with order-only (sync=False) deps.
                _prev = {}

                def _pin(key, bi):
                    p = _prev.get(key)
                    if p is not None:
                        tile.add_dep_helper(
                            bi.ins, p.ins, sync=False, reason="pin order"
                        )
                    _prev[key] = bi
                    return bi

                def _mm(*args, **kwargs):
                    return _pin("pe", nc.tensor.matmul(*args, **kwargs))

                def _tr(*args, **kwargs):
                    return _pin("pe", nc.tensor.transpose(*args, **kwargs))

                def _dve(bi):
                    return _pin("dve", bi)

                def _act(bi):
                    return _pin("act", bi)

                def _dummy_mms(k):
                    for _ in range(k):
                        _mm(e1kv[DE:, :], lhsT=warm_sb[:, :DE], rhs=warm_sb)

                # Load the sigmoid activation table up front (the junk
                # sigmoid also primes the Scalar pipeline).
                warm_o = small.tile([1, 4], f16)
                _act(nc.scalar.activation(
                    warm_o, warm_sb[:1, :4], mybir.ActivationFunctionType.Sigmoid
                ))
                # One CONTIGUOUS keep-alive burst (~4.3us at the cold clock).
                # The HAM clock gate only un-throttles after a full
                # 4096-cycle window of sustained PE activity; fragmented
                # fills (interleaved with data waits) leave the PE at
                # 1.2 GHz for 15-22us.  The burst also bridges engine-ready
                # (~7.4us) to weights+first-data (~8.7us).
                _dummy_mms(10)
                state = {}  # j -> (e23t_sb | e23n) between pipeline stages

                def _transpose_part(j):
                    # transpose e23T back to natural layout (batched into one
                    # PSUM bank); the drain copy runs on the Scalar engine --
                    # DVE keeps only the bias-add so neither engine exceeds
                    # the ~1.25us chunk cadence.
                    e23t_sb = state.pop(j)
                    tr_ps = trpsp.tile([128, 512], f16)
                    for t in range(4):
                        _tr(
                            tr_ps[:, t * 128 : (t + 1) * 128],
                            e23t_sb[:, t * 128 : (t + 1) * 128],
                            identity[:, :],
                        )
                    e23n = e23np.tile([128, 512], f16)
                    _act(nc.scalar.copy(e23n, tr_ps))
                    state[j] = e23n

                def _kv_part(j):
                    # accumulate KV^T = e3^T @ e2.  The last two chunks go to
                    # the second accumulator so the first (dominant) KV part
                    # can head into M early.
                    e23n = state.pop(j)
                    last2 = j >= n_chunks - 2
                    dst = kvb_ps if last2 else kvt_ps
                    lo = (n_chunks - 2) * 4 if last2 else 0
                    hi = 4 * n_chunks - 1 if last2 else (n_chunks - 2) * 4 - 1
                    for t in range(4):
                        tt = j * 4 + t
                        _mm(
                            dst,
                            lhsT=e23n[:, t * 128 + DE : (t + 1) * 128],
                            rhs=e23n[:, t * 128 : t * 128 + DE],
                            start=(tt == lo),
                            stop=(tt == hi),
                        )

                def _front_half(j, xr):
                    # e23T = [W2;W3] @ x^T  -> [128, 512] (d on partitions)
                    e23t_ps = e23tpsp.tile([128, 512], f32)
                    for kt in range(4):
                        _mm(
                            e23t_ps,
                            lhsT=sb_w23t[:, kt, :],
                            rhs=xr[:, kt, :],
                            start=(kt == 0),
                            stop=(kt == 3),
                        )
                    # bias add + f16 cast on DVE
                    e23t_sb = e23tsbp.tile([128, 512], f16)
                    _dve(nc.vector.tensor_scalar_add(e23t_sb, e23t_ps, sb_b23))
                    state[j] = e23t_sb

                    # e1T = W1 @ x^T (+b1) for the rows this core outputs.
                    # Own chunks ride at positions 2..5: early enough that e1
                    # never lands on the post-stream tail, late enough that
                    # the cold-clock window (first ~2 chunks) only covers
                    # single-size chunks.
                    if OWN0 <= j < OWN0 + own_chunks:
                        jo = j - OWN0
                        e1_ps = e1kv[:DE, :]
                        for kt in range(4):
                            _mm(
                                e1_ps,
                                lhsT=sb_w1t[:, kt, :],
                                rhs=xr[:, kt, :],
                                start=(kt == 0),
                                stop=(kt == 3),
                            )
                        _dve(nc.vector.tensor_scalar_add(
                            e1t[:DE, jo * 512 : (jo + 1) * 512], e1_ps, sb_b1
                        ))

                # Single-chunk DMAs throughout: DMA completion is
                # all-or-nothing per instruction, so a 4-chunk (2MB) group
                # would stall the consumer of its FIRST chunk for an extra
                # ~4.4us.  512KB transfers still run at ~350 GB/s.
                # (Splitting the stream across the sync+scalar HWDGE rings
                # was tried and measured ~5us WORSE: the scalar-ring issues
                # interleave with the e23n copy chain.)
                schedule = [1] * n_chunks
                assert sum(schedule) == n_chunks
                # M = KV @ WO.T accumulates in the e1 region of the shared
                # bank -- e1 is long finished (own chunks come first).
                mm_ps = e1kv[:DE, :]
                j = 0
                for g in schedule:
                    xf = xfp.tile([128, g, 2048], f16, tag="xf")
                    if j == 0:
                        # quarter-granularity leading DMAs: the first e23T
                        # matmul only needs the first 512 columns
                        for kt in range(4):
                            nc.sync.dma_start(
                                out=xf[:, 0, kt * 512 : (kt + 1) * 512],
                                in_=xt_t[:, 0, kt * 512 : (kt + 1) * 512],
                            )
                    else:
                        nc.sync.dma_start(
                            out=xf[:, :g, :], in_=xt_t[:, j : j + g, :]
                        )
                    for j2 in range(g):
                        xr = xf[:, j2, :].rearrange("p (kt s) -> p kt s", kt=4)
                        # matmuls first, then transpose(j-1), then KV(j-2):
                        # the PE is strictly in-order, so the KV stage needs
                        # TWO chunks of lag for the transpose -> Scalar-copy
                        # round trip to finish off the critical path.
                        _front_half(j, xr)
                        if j >= 1:
                            _transpose_part(j - 1)
                        if j >= 2:
                            _kv_part(j - 2)
                        if j == 3:
                            # release the tail-only blob half only after the
                            # early ramp: it must not compete with W23T / the
                            # first x chunks for DMA bandwidth
                            tile.add_dep_helper(
                                blob_rest.ins, _prev["pe"].ins,
                                sync=True, reason="delay tail blob",
                            )
                        j += 1
                # Tail peel.  KV runs at lag 2, so kvt (chunks 0..n-3)
                # completed inside the loop; its SBUF copy overlaps the
                # remaining PE work so M part 1 is ready just in time.
                kvt_r = small.tile([DE, DE], f16)
                _dve(nc.vector.tensor_copy(kvt_r, kvt_ps))
                _transpose_part(n_chunks - 1)
                _kv_part(n_chunks - 2)
                _mm(mm_ps, lhsT=kvt_r, rhs=sb_wot, start=True, stop=False)
                _kv_part(n_chunks - 1)
                kvb_r = small.tile([DE, DE], f16)
                _dve(nc.vector.tensor_copy(kvb_r, kvb_ps))
                _dve(nc.vector.tensor_copy(mmat[DE : DE + 1, :], sb_bo))
                _dummy_mms(2)
                _mm(mm_ps, lhsT=kvb_r, rhs=sb_wot, start=False, stop=True)
                _dummy_mms(2)
                _dve(nc.vector.tensor_copy(mmat[:DE, :], mm_ps))

            # ---- Phase C: out = sigmoid(e1 @ M + bO) ----
            # z-tiles land as f32 in PSUM (one bank each, 4 banks per pool
            # tile); the sigmoid runs on Scalar DIRECTLY from PSUM over
            # 4-tile batches -- no DVE casts (ACTIVATE is 1x rate regardless
            # of dtype, so the old CAST+ACT chain only added DVE
            # serialization).  The PE issues all 16 z matmuls back-to-back
            # at the warm clock.
            with (
                tc.tile_pool(name="zps", bufs=2, space="PSUM") as zpsp,
                tc.tile_pool(name="osb", bufs=3) as osbp,
            ):
                out_flat = out.ap()
                nbat = rows // 128 // 4  # 4 z-tiles (= 4 PSUM banks) per batch
                for p in range(nbat):
                    z_ps = zpsp.tile([128, 4, DIN], f32, tag="zps")
                    for i in range(4):
                        t = 4 * p + i
                        nc.tensor.matmul(
                            z_ps[:, i, :],
                            lhsT=e1t[: DE + 1, t * 128 : (t + 1) * 128],
                            rhs=mmat[: DE + 1, :],
                        )
                    osb = osbp.tile([128, 4, DIN], f16, tag="osb")
                    # all out-DMAs ride the (idle) sync ring -- issuing from
                    # the scalar ring put a ~650ns DMA-issue in the middle of
                    # the sigmoid chain, which is the tail critical path.
                    # The last batch's sigmoid is split 3+1 so the final
                    # flush after the very last sigmoid is only ~128KB.
                    if p < nbat - 1:
                        nc.scalar.activation(
                            osb, z_ps, mybir.ActivationFunctionType.Sigmoid
                        )
                        nc.sync.dma_start(
                            out=out_flat[:, p * 2048 : (p + 1) * 2048],
                            in_=osb,
                        )
                    else:
                        nc.scalar.activation(
                            osb[:, :3, :], z_ps[:, :3, :],
                            mybir.ActivationFunctionType.Sigmoid,
                        )
                        nc.sync.dma_start(
                            out=out_flat[:, p * 2048 : p * 2048 + 1536],
                            in_=osb[:, 0:3, :],
                        )
                        nc.scalar.activation(
                            osb[:, 3:, :], z_ps[:, 3:, :],
                            mybir.ActivationFunctionType.Sigmoid,
                        )
                        nc.sync.dma_start(
                            out=out_flat[:, p * 2048 + 1536 : (p + 1) * 2048],
                            in_=osb[:, 3:4, :],
                        )
    nc.compile()
    return nc


def make_wconst(W1, b1, W2, b2, W3, b3, WO, bO):
    blob = np.zeros((128, _NB), np.float16)
    w1t = np.asarray(W1, np.float16).T.reshape(4, 128, DE)  # (kt, p, d)
    blob[:, _OFF_W1T : _OFF_W1T + 256] = (
        w1t.transpose(1, 0, 2).reshape(128, 4 * DE)
    )
    w23t = np.concatenate(
        [np.asarray(W2, np.float16).T, np.asarray(W3, np.float16).T], axis=1
    ).reshape(4, 128, 2 * DE)
    blob[:, _OFF_W23T : _OFF_W23T + 512] = (
        w23t.transpose(1, 0, 2).reshape(128, 8 * DE)
    )
    blob[:, _OFF_IDENT : _OFF_IDENT + 128] = np.eye(128, dtype=np.float16)
    blob[:, _OFF_B23] = np.concatenate(
        [np.asarray(b2, np.float16), np.asarray(b3, np.float16)]
    )
    blob[:DE, _OFF_WOT : _OFF_WOT + DIN] = np.asarray(WO, np.float16).T
    blob[:DE, _OFF_B1] = np.asarray(b1, np.float16)
    blob[0, _OFF_BO : _OFF_BO + DIN] = np.asarray(bO, np.float16)
    return blob


def _tile_rows(xc):
    """[rows, 512] f16 -> [128, (rows/512)*2048] in (p, chunk, kt, s) order."""
    n = xc.shape[0] // 512
    return np.ascontiguousarray(
        xc.reshape(n, 512, 4, 128).transpose(3, 0, 2, 1)
    ).reshape(128, n * 2048)


def make_in_maps(x, W1, b1, W2, b2, W3, b3, WO, bO, rows=ROWS, n_cores=N_CORES):
    x = np.asarray(x, dtype=np.float32).astype(np.float16)
    total = x.shape[0] * x.shape[1]
    xf = x.reshape(total, DIN)
    blob = make_wconst(W1, b1, W2, b2, W3, b3, WO, bO)
    bvec = np.zeros((128, 2), np.float32)
    bvec[:, 0] = np.concatenate([np.asarray(b2, np.float32), np.asarray(b3, np.float32)])
    bvec[:DE, 1] = np.asarray(b1, np.float32)
    group = n_cores // 2
    batch_rows = rows * group
    in_maps = []
    for c in range(n_cores):
        b, q = divmod(c, group)
        xb = xf[b * batch_rows : (b + 1) * batch_rows]  # full batch of this core
        own = xb[q * rows : (q + 1) * rows]
        rest = np.concatenate([xb[: q * rows], xb[(q + 1) * rows :]], axis=0)
        cut = OWN0 * 512  # own chunks sit at stream positions OWN0..OWN0+3
        m = {
            "wconst": blob,
            "bias32": bvec,
            "xt": np.concatenate(
                [
                    _tile_rows(rest[:cut]),
                    _tile_rows(own),
                    _tile_rows(rest[cut:]),
                ],
                axis=1,
            ),
        }
        in_maps.append(m)
    return in_maps


def unshard_out(o, rows=ROWS):
    # o: [128, rows*4] f16 laid out (p, j, t, o) -> rows j*512 + t*128 + p
    n_chunks = rows // 512
    return (
        o.astype(np.float32)
        .reshape(128, n_chunks, 4, DIN)
        .transpose(1, 2, 0, 3)
        .reshape(rows, DIN)
    )


def kernel(x, W1, b1, W2, b2, W3, b3, WO, bO):
    global LAST_RESULT
    if "nc" not in _NC_CACHE:
        _NC_CACHE["nc"] = build_nc()
    nc = _NC_CACHE["nc"]
    in_maps = make_in_maps(x, W1, b1, W2, b2, W3, b3, WO, bO)
    res = run_bass_kernel_spmd(
        nc,
        in_maps,
        core_ids=list(range(N_CORES)),
        trace=TRACE,
        **TRACE_KWARGS,
    )
    LAST_RESULT = res
    full = np.concatenate(
        [unshard_out(res.results[c]["out"]) for c in range(N_CORES)], axis=0
    )  # [16384, 512] f32
    return full.reshape(BATCH, SEQ, DIN)
